# revision 8
# baseline (speedup 1.0000x reference)
"""Trainium2 Bass kernel for nn_Attention_41704132444382.

Masked-linear QKV projection + 16-head attention + masked-linear output
projection, tensor-parallel over heads across 8 NeuronCores (2 heads/core).

v2 design (ScalarE-exp is the roofline: ~128us of exp streaming):
  - Host: gates both masked-linear weights (sigmoid(m)>0.5), transposes x,
    casts x / wqkv / wo to bf16 (wqkv/wo values are +-c, near-exact in bf16).
  - QKV: xq bf16 tiles [128, 4096] x 8 kt-chunks; lhsT = gated wqkv bf16
    (FWL weight loads); psum [128,512] chains; q/k evacuated by ScalarE
    (Copy -> f32r), v by DVE (cast -> bf16). V^T PE-transposed (bf16) to
    v1/v2 [t, dv|1] tiles with a ones column at stride 65 (PV then yields
    attn-out^T AND the softmax denominator in one accumulation chain).
  - Attention per 1024-query block, h-offset pipeline: per key-tile jt,
    S^T = kT.T @ qT per head into s_h [128,1024] (2 psum banks); one
    1024-wide exp ACT per head (scale=1/32) -> e_h bf16; PV lags LAG
    key-tiles behind (e-ring depth covers it) so block-boundary work can
    drain the pv psum rings without stalling ScalarE.
  - Softmax denominators: pv row 64 -> [1,1024] copy, DVE reciprocal
    (f32r out), broadcast to [64,1024] via K=1 ones-matmul, normalize
    attnT with one tensor_tensor per head.
  - Output projection: lhsT = attnT bf16 (FWL), po pairs [128,1024] in
    the pv psum rings at block boundaries, DVE evac, DMA from SBUF.
"""

import os
import sys

import numpy as np

sys.path.insert(0, "/opt/trn_rl_repo")

import concourse.bass as bass
import concourse.mybir as mybir
from concourse import bacc
from concourse.masks import make_identity
from concourse.tile import TileContext

DIM = 1024
HEADS = 16
B = 2
N = 2048
T = B * N  # 4096 flattened tokens
NCORES = 8
HPC = HEADS // NCORES  # 2 heads per core
DV = HPC * 64  # 128 head-dims per core
SCALE = DIM ** (-0.5)  # 1/32
# PV runs LAG key-tiles behind exp so block-boundary work (norm broadcast +
# out-projection) can drain through the pv psum rings without stalling the
# ScalarE exp pipeline; the e-ring depth covers the lag.
LAG = 7

F32 = mybir.dt.float32
F32R = mybir.dt.float32r
BF16 = mybir.dt.bfloat16

Copy = mybir.ActivationFunctionType.Copy
Exp = mybir.ActivationFunctionType.Exp
mult = mybir.AluOpType.mult


def build_nc():
    nc = bacc.Bacc("TRN2", target_bir_lowering=True)
    xT_d = nc.declare_dram_parameter("xT", [DIM, T], BF16, isOutput=False)
    wqkvT_d = nc.declare_dram_parameter("wqkvT", [DIM, 384], BF16, isOutput=False)
    woT_d = nc.declare_dram_parameter("woT", [DV, DIM], BF16, isOutput=False)
    out_d = nc.declare_dram_parameter("out", [T, DIM], F32, isOutput=True)

    with TileContext(nc) as tc:
        with tc.tile_pool(name="persist", bufs=1) as pp:
            qT = pp.tile([128, T], F32R)
            kTt = pp.tile([128, T], F32R)
            v1 = pp.tile([128, 32 * 65], BF16)  # [t-part, (jt, dv|1)] head 0
            v2 = pp.tile([128, 32 * 65], BF16)  # head 1
            attnT = pp.tile([128, T], BF16)  # [dv-part, t] normalized
            wo_g = pp.tile([128, DIM], BF16)
            ident = pp.tile([128, 128], BF16)
            ones1 = pp.tile([1, 64], F32R)

            make_identity(nc, ident[:])
            ones_f = pp.tile([128, 64], F32)
            nc.vector.memset(ones_f[:], 1.0)
            nc.vector.tensor_copy(ones1[:], ones_f[0:1, :])
            ones32 = pp.tile([128, 32], BF16)
            nc.vector.tensor_copy(ones32[:], ones_f[:, 0:32])
            # ones column at slot 64 of each 65-wide block of v1/v2; V
            # evacuations only write cols 0..63 of each block.
            for vv in (v1, v2):
                nc.vector.tensor_copy(
                    vv[:].rearrange("p (j c) -> p j c", c=65)[:, :, 64:65],
                    ones32[:].rearrange("p (j c) -> p j c", c=1),
                )
            # preload the exp activation table while DMAs run
            junk = pp.tile([1, 32], F32)
            nc.vector.memset(junk[:], 0.0)
            junk2 = pp.tile([1, 32], F32)
            nc.scalar.activation(junk2[:], junk[:], Exp)

            nc.sync.dma_start(wo_g[:], woT_d[:])

            # ---------- Phase 1: QKV projection (+ V^T transpose) ----------
            with (
                tc.tile_pool(name="ph1", bufs=1) as p1,
                tc.tile_pool(name="qkv_ps", bufs=4, space="PSUM") as qkps,
            ):
                wqkv_g = p1.tile([128, 8 * 384], BF16)  # [k-part, (kt, o)]
                nc.sync.dma_start(
                    wqkv_g[:].rearrange("p (kt o) -> p kt o", kt=8),
                    wqkvT_d[:].rearrange("(kt p) o -> p kt o", p=128),
                )
                xq = [p1.tile([128, T], BF16, name=f"xq{i}") for i in range(8)]
                vT = p1.tile([128, T], BF16)
                # x arrives in t-quarter chunks so quarter-0 compute starts
                # after ~1/4 of the x traffic
                dmae = [nc.sync, nc.gpsimd]
                for q in range(4):
                    for kt in range(8):
                        dmae[kt % 2].dma_start(
                            xq[kt][:, q * 1024 : (q + 1) * 1024],
                            xT_d[kt * 128 : (kt + 1) * 128, q * 1024 : (q + 1) * 1024],
                        )

                for q in range(4):
                    # v first so transposes can interleave with q/k matmuls
                    for ot, dest in ((2, vT), (1, kTt), (0, qT)):
                        for th in range(2):
                            ps = qkps.tile([128, 512], F32, tag="qk")
                            for kt in range(8):
                                nc.tensor.matmul(
                                    ps[:],
                                    wqkv_g[:, kt * 384 + ot * 128 : kt * 384 + (ot + 1) * 128],
                                    xq[kt][:, q * 1024 + th * 512 : q * 1024 + (th + 1) * 512],
                                    start=(kt == 0),
                                    stop=(kt == 7),
                                )
                            col = q * 1024 + th * 512
                            if ot == 2:
                                nc.vector.tensor_copy(vT[:, col : col + 512], ps[:])
                            else:
                                nc.scalar.activation(
                                    dest[:, col : col + 512], ps[:], Copy
                                )
                    for tj in range(8):  # V^T -> v1/v2 for this quarter
                        jt = q * 8 + tj
                        ptv = qkps.tile([128, 128], BF16, tag="vt", bufs=2)
                        nc.tensor.transpose(
                            ptv[:], vT[:, jt * 128 : (jt + 1) * 128], ident[:]
                        )
                        nc.vector.tensor_copy(v1[:, jt * 65 : jt * 65 + 64], ptv[:, 0:64])
                        nc.vector.tensor_copy(v2[:, jt * 65 : jt * 65 + 64], ptv[:, 64:128])

            # ---------- Phase 2: attention ----------
            with (
                tc.tile_pool(name="esb", bufs=1) as ep,
                tc.tile_pool(name="small", bufs=1) as sp,
                tc.tile_pool(name="osb", bufs=1) as osp,
                tc.tile_pool(name="s_ps", bufs=1, space="PSUM") as sps,
                tc.tile_pool(name="pv_ps", bufs=1, space="PSUM") as pvps,
            ):
                blocks = [(b, ib) for b in range(B) for ib in range(2)]

                def emit_boundary(pb, pib, step):
                    """Norm broadcast + out-projection for block (pb, pib),
                    interleaved into the next block's jt loop (or flushed at
                    the end).  step 0/1: rbc+normalize per head; 2..5: two
                    po pairs each."""
                    i0 = pb * 2048 + pib * 1024
                    key = f"{pb}_{pib}"
                    if step < 2:
                        h = step
                        rbc = pvps.tile(
                            [64, 1024], F32, tag=f"pv{h}", name=f"rbc{key}_{h}"
                        )
                        for ih in range(2):
                            nc.tensor.matmul(
                                rbc[:, ih * 512 : (ih + 1) * 512],
                                ones1[:],
                                rcp[key][h][0:1, ih * 512 : (ih + 1) * 512],
                                start=True,
                                stop=True,
                            )
                        rbs = sp.tile([64, 1024], F32, tag=f"rbs{h}", name=f"rbs{key}_{h}")
                        nc.vector.tensor_copy(rbs[:], rbc[:])
                        nc.vector.tensor_tensor(
                            attnT[h * 64 : (h + 1) * 64, i0 : i0 + 1024],
                            unorm[key][h][:],
                            rbs[:],
                            mult,
                        )
                    else:
                        for k in range(2):
                            tg = (step - 2) * 2 + k
                            row = i0 + tg * 128
                            po = pvps.tile(
                                [128, 1024], F32, tag=f"pv{tg % 2}", name=f"po{key}_{tg}"
                            )
                            for oh in range(2):
                                nc.tensor.matmul(
                                    po[:, oh * 512 : (oh + 1) * 512],
                                    attnT[:, row : row + 128],
                                    wo_g[:, oh * 512 : (oh + 1) * 512],
                                    start=True,
                                    stop=True,
                                )
                            ob = osp.tile(
                                [128, 1024], F32, tag="ob", bufs=4, name=f"ob{key}_{tg}"
                            )
                            nc.vector.tensor_copy(ob[:], po[:])
                            dmae = nc.sync if tg % 2 == 0 else nc.gpsimd
                            dmae.dma_start(out_d[row : row + 128, :], ob[:])

                unorm = {}
                rcp = {}
                prev = None
                for b, ib in blocks:
                    key = f"{b}_{ib}"
                    i0 = b * 2048 + ib * 1024
                    # allocated lazily at the first emit_pv so the pv-ring
                    # order is: prev block's pv -> prev's rbc/po -> ours
                    pv = []
                    e_pend = []

                    def emit_pv(jt, key=key, b=b, pv=pv, e_pend=e_pend):
                        if not pv:
                            pv.extend(
                                pvps.tile([65, 1024], F32, tag=f"pv{h}", name=f"pv{key}_{h}")
                                for h in range(2)
                            )
                        eh = e_pend.pop(0)
                        jv = (b * 16 + jt) * 65
                        for h, vv in enumerate((v1, v2)):
                            for ih in range(2):
                                nc.tensor.matmul(
                                    pv[h][:, ih * 512 : (ih + 1) * 512],
                                    vv[:, jv : jv + 65],
                                    eh[h][:, ih * 512 : (ih + 1) * 512],
                                    start=(jt == 0),
                                    stop=(jt == 15),
                                )

                    for jt in range(16):
                        j0 = b * 2048 + jt * 128
                        s_h = [
                            sps.tile([128, 1024], F32, tag=f"s{h}", name=f"s{key}_{jt}_{h}")
                            for h in range(2)
                        ]
                        e_h = [
                            ep.tile([128, 1024], BF16, tag=f"e{h}", bufs=LAG + 2,
                                    name=f"e{key}_{jt}_{h}")
                            for h in range(2)
                        ]
                        # S pairs interleaved over heads: (0,0)/(64,0) row
                        # tiles can run concurrently on the PE
                        for ih in range(2):
                            for h in range(2):
                                nc.tensor.matmul(
                                    s_h[h][:, ih * 512 : (ih + 1) * 512],
                                    kTt[h * 64 : (h + 1) * 64, j0 : j0 + 128],
                                    qT[h * 64 : (h + 1) * 64, i0 + ih * 512 : i0 + (ih + 1) * 512],
                                    start=True,
                                    stop=True,
                                )
                        for h in range(2):
                            nc.scalar.activation(e_h[h][:], s_h[h][:], Exp, scale=SCALE)
                        e_pend.append(e_h)
                        if len(e_pend) > LAG:
                            emit_pv(jt - LAG)
                        # previous block's norm + out-projection, spread out
                        if prev is not None and 1 <= jt <= 6:
                            emit_boundary(prev[0], prev[1], jt - 1)
                    for jt in range(16 - LAG, 16):
                        emit_pv(jt)

                    # denominators first (the next block's rbc matmuls wait
                    # on them early), then the unnormalized evacuations
                    rcp[key] = []
                    for h in range(2):
                        cs = sp.tile([1, 1024], F32, tag=f"cs{h}", name=f"cs{key}_{h}")
                        nc.vector.tensor_copy(cs[:], pv[h][64:65, :])
                        r = sp.tile([1, 1024], F32R, tag=f"r{h}", name=f"r{key}_{h}")
                        # f32r is 32-bit storage; only the dtype tag trips the guard
                        with nc.allow_low_precision(reason="f32r is fp32 storage"):
                            nc.vector.reciprocal(r[:], cs[:])
                        rcp[key].append(r)
                    unorm[key] = [
                        sp.tile([64, 1024], F32, tag=f"un{h}", name=f"un{key}_{h}")
                        for h in range(2)
                    ]
                    for h in range(2):
                        nc.vector.tensor_copy(unorm[key][h][:], pv[h][0:64, :])
                    prev = (b, ib)

                # flush the last block's norm + out-projection
                for step in range(6):
                    emit_boundary(prev[0], prev[1], step)

    nc.compile()
    return nc


_NC = None


def _get_nc():
    global _NC
    if _NC is None:
        _NC = build_nc()
    return _NC


def _gate(mask):
    """Exact jax fp32 gate: sigmoid(m) > 0.5 (matches reference rounding)."""
    mask = np.asarray(mask, dtype=np.float32)
    return (np.float32(1.0) / (np.float32(1.0) + np.exp(-mask))) > np.float32(0.5)


def make_in_maps(x, qkv_weight, qkv_weight_mask, out_weight, out_weight_mask):
    import ml_dtypes

    bf16 = ml_dtypes.bfloat16
    x = np.asarray(x, dtype=np.float32)
    wqkv = np.where(_gate(qkv_weight_mask), np.asarray(qkv_weight, np.float32), 0.0)
    wo = np.where(_gate(out_weight_mask), np.asarray(out_weight, np.float32), 0.0)

    xT = np.ascontiguousarray(x.reshape(T, DIM).T).astype(bf16)
    in_maps = []
    for c in range(NCORES):
        r0 = c * DV
        sl = slice(r0, r0 + DV)
        w_shard = np.concatenate(
            [wqkv[sl], wqkv[DIM + r0 : DIM + r0 + DV], wqkv[2 * DIM + r0 : 2 * DIM + r0 + DV]],
            axis=0,
        )  # [384, 1024] rows = (q | k | v) for this core's 2 heads
        in_maps.append(
            {
                "xT": xT,
                "wqkvT": np.ascontiguousarray(w_shard.T).astype(bf16),
                "woT": np.ascontiguousarray(wo[:, sl].T).astype(bf16),
            }
        )
    return in_maps


LAST_RESULTS = None  # BassKernelResults of the most recent run (for profiling)


def kernel(
    x,
    qkv_weight,
    qkv_weight_mask,
    out_weight,
    out_weight_mask,
    out_bias,
    out_bias_mask,
    _trace=False,
    _tmpdir=None,
):
    global LAST_RESULTS
    from concourse.bass_utils import run_bass_kernel_spmd

    nc = _get_nc()
    in_maps = make_in_maps(x, qkv_weight, qkv_weight_mask, out_weight, out_weight_mask)
    res = run_bass_kernel_spmd(
        nc, in_maps, list(range(NCORES)), trace=_trace, tmpdir=_tmpdir
    )
    LAST_RESULTS = res
    out = np.zeros((T, DIM), dtype=np.float32)
    for r in res.results:
        out += r["out"]
    out_bias = np.asarray(out_bias, dtype=np.float32)
    out += np.where(_gate(out_bias_mask), out_bias, np.float32(0.0))[None, :]
    return out.reshape(B, N, DIM)


# revision 24
# speedup vs baseline: 1.5407x; 1.5407x over previous
"""Trainium2 Bass kernel for nn_Attention_41704132444382.

Masked-linear QKV projection + 16-head attention + masked-linear output
projection, tensor-parallel over heads across 8 NeuronCores (2 heads/core).

v2 design (ScalarE-exp is the roofline: ~128us of exp streaming):
  - Host: gates both masked-linear weights (sigmoid(m)>0.5), transposes x,
    casts x / wqkv / wo to bf16 (wqkv/wo values are +-c, near-exact in bf16).
  - QKV: xq bf16 tiles [128, 4096] x 8 kt-chunks; lhsT = gated wqkv bf16
    (FWL weight loads); psum [128,512] chains; q/k evacuated by ScalarE
    (Copy -> f32r), v by DVE (cast -> bf16). V^T PE-transposed (bf16) to
    v1/v2 [t, dv|1] tiles with a ones column at stride 65 (PV then yields
    attn-out^T AND the softmax denominator in one accumulation chain).
  - Attention per 1024-query block, h-offset pipeline: per key-tile jt,
    S^T = kT.T @ qT per head into s_h [128,1024] (2 psum banks); one
    1024-wide exp ACT per head (scale=1/32) -> e_h bf16; PV lags LAG
    key-tiles behind (e-ring depth covers it) so block-boundary work can
    drain the pv psum rings without stalling ScalarE.
  - Softmax denominators: pv row 64 -> [1,1024] copy, DVE reciprocal
    (f32r out), broadcast to [64,1024] via K=1 ones-matmul, normalize
    attnT with one tensor_tensor per head.
  - Output projection: lhsT = attnT bf16 (FWL), po pairs [128,1024] in
    the pv psum rings at block boundaries, DVE evac, DMA from SBUF.
"""

import os
import sys

import numpy as np

sys.path.insert(0, "/opt/trn_rl_repo")

import concourse.bass as bass
import concourse.mybir as mybir
from concourse import bacc
from concourse.masks import make_identity
from concourse.tile import TileContext

DIM = 1024
HEADS = 16
B = 2
N = 2048
T = B * N  # 4096 flattened tokens
NCORES = 8
HPC = HEADS // NCORES  # 2 heads per core
DV = HPC * 64  # 128 head-dims per core
SCALE = DIM ** (-0.5)  # 1/32
# PV runs LAG key-tiles behind exp so block-boundary work (norm broadcast +
# out-projection) can drain through the pv psum rings without stalling the
# ScalarE exp pipeline; the e-ring depth covers the lag.
LAG = 7

F32 = mybir.dt.float32
F32R = mybir.dt.float32r
BF16 = mybir.dt.bfloat16

Copy = mybir.ActivationFunctionType.Copy
Exp = mybir.ActivationFunctionType.Exp
mult = mybir.AluOpType.mult


def build_nc():
    nc = bacc.Bacc("TRN2", target_bir_lowering=True)
    xT_d = nc.declare_dram_parameter("xT", [DIM, T], BF16, isOutput=False)
    wqkvT_d = nc.declare_dram_parameter("wqkvT", [DIM, 384], BF16, isOutput=False)
    woT_d = nc.declare_dram_parameter("woT", [DV, DIM], BF16, isOutput=False)
    out_d = nc.declare_dram_parameter("out", [T, DIM], F32, isOutput=True)

    with TileContext(nc) as tc:
        with tc.tile_pool(name="persist", bufs=1) as pp:
            # S runs as K=128 all-bf16 matmuls (FWL weight loads keep PE array
            # duty high -> HAM stays at 2.4 GHz) with split-precision q in the
            # otherwise-idle half of the array: qs rows 0-63 = bf16(q), rows
            # 64-127 = bf16(q - bf16(q)); ks duplicates k in both halves, so
            # k.T q accumulates the hi and lo products -> q is fp32-exact.
            qs = [pp.tile([128, T], BF16, name=f"qs{h}") for h in range(HPC)]
            ks = [pp.tile([128, T], BF16, name=f"ks{h}") for h in range(HPC)]
            v1 = pp.tile([128, 32 * 65], BF16)  # [t-part, (jt, dv|1)] head 0
            v2 = pp.tile([128, 32 * 65], BF16)  # head 1
            attnT = pp.tile([128, T], BF16)  # [dv-part, t] normalized
            wo_g = pp.tile([128, DIM], BF16)
            ident = pp.tile([128, 128], BF16)
            identf = pp.tile([128, 128], F32)
            ones1 = pp.tile([1, 64], F32R)

            make_identity(nc, ident[:])
            make_identity(nc, identf[:])
            ones_f = pp.tile([128, 64], F32)
            nc.vector.memset(ones_f[:], 1.0)
            nc.vector.tensor_copy(ones1[:], ones_f[0:1, :])
            ones32 = pp.tile([128, 32], BF16)
            nc.vector.tensor_copy(ones32[:], ones_f[:, 0:32])
            # ones column at slot 64 of each 65-wide block of v1/v2; V
            # evacuations only write cols 0..63 of each block.
            for vv in (v1, v2):
                nc.vector.tensor_copy(
                    vv[:].rearrange("p (j c) -> p j c", c=65)[:, :, 64:65],
                    ones32[:].rearrange("p (j c) -> p j c", c=1),
                )
            # preload the exp activation table while DMAs run
            junk = pp.tile([1, 32], F32)
            nc.vector.memset(junk[:], 0.0)
            junk2 = pp.tile([1, 32], F32)
            nc.scalar.activation(junk2[:], junk[:], Exp)

            nc.sync.dma_start(wo_g[:], woT_d[:])

            # ---------- Phase 1: QKV projection (+ V^T transpose) ----------
            with (
                tc.tile_pool(name="ph1", bufs=1) as p1,
                tc.tile_pool(name="qkv_ps", bufs=4, space="PSUM") as qkps,
            ):
                wqkv_g = p1.tile([128, 8 * 384], BF16)  # [k-part, (kt, o)]
                nc.sync.dma_start(
                    wqkv_g[:].rearrange("p (kt o) -> p kt o", kt=8),
                    wqkvT_d[:].rearrange("(kt p) o -> p kt o", p=128),
                )
                xq = [p1.tile([128, T], BF16, name=f"xq{i}") for i in range(8)]
                vT = p1.tile([128, T], BF16)
                # x arrives in t-quarter chunks so quarter-0 compute starts
                # after ~1/4 of the x traffic
                dmae = [nc.sync, nc.gpsimd, nc.scalar]
                for q in range(4):
                    for kt in range(8):
                        dmae[kt % 3].dma_start(
                            xq[kt][:, q * 1024 : (q + 1) * 1024],
                            xT_d[kt * 128 : (kt + 1) * 128, q * 1024 : (q + 1) * 1024],
                        )

                sub = mybir.AluOpType.subtract
                for q in range(4):
                    # v first so transposes can interleave with q/k matmuls
                    for ot in (2, 1, 0):
                        for th in range(2):
                            ps = qkps.tile([128, 512], F32, tag="qk")
                            for kt in range(8):
                                nc.tensor.matmul(
                                    ps[:],
                                    wqkv_g[:, kt * 384 + ot * 128 : kt * 384 + (ot + 1) * 128],
                                    xq[kt][:, q * 1024 + th * 512 : q * 1024 + (th + 1) * 512],
                                    start=(kt == 0),
                                    stop=(kt == 7),
                                )
                            col = q * 1024 + th * 512
                            cs_ = slice(col, col + 512)
                            if ot == 2:
                                nc.vector.tensor_copy(vT[:, cs_], ps[:])
                            elif ot == 0:  # q: hi = bf16(q), lo = q - hi
                                for hh in range(2):
                                    php = ps[hh * 64 : (hh + 1) * 64, :]
                                    nc.scalar.activation(qs[hh][0:64, cs_], php, Copy)
                                    nc.vector.tensor_tensor(
                                        qs[hh][64:128, cs_], php, qs[hh][0:64, cs_], sub
                                    )
                            else:  # k: duplicated into both array halves
                                for hh in range(2):
                                    php = ps[hh * 64 : (hh + 1) * 64, :]
                                    nc.scalar.activation(ks[hh][0:64, cs_], php, Copy)
                                    nc.vector.tensor_copy(ks[hh][64:128, cs_], ks[hh][0:64, cs_])
                    for tj in range(8):  # V^T -> v1/v2 for this quarter
                        jt = q * 8 + tj
                        ptv = qkps.tile([128, 128], BF16, tag="vt", bufs=2)
                        nc.tensor.transpose(
                            ptv[:], vT[:, jt * 128 : (jt + 1) * 128], ident[:]
                        )
                        nc.vector.tensor_copy(v1[:, jt * 65 : jt * 65 + 64], ptv[:, 0:64])
                        nc.vector.tensor_copy(v2[:, jt * 65 : jt * 65 + 64], ptv[:, 64:128])

            # ---------- Phase 2: attention ----------
            with (
                tc.tile_pool(name="esb", bufs=1) as ep,
                tc.tile_pool(name="small", bufs=1) as sp,
                tc.tile_pool(name="osb", bufs=1) as osp,
                tc.tile_pool(name="s_ps", bufs=1, space="PSUM") as sps,
                tc.tile_pool(name="pv_ps", bufs=1, space="PSUM") as pvps,
            ):
                blocks = [(b, ib) for b in range(B) for ib in range(2)]

                def emit_boundary(pb, pib, step):
                    """Norm + out-projection for block (pb, pib), interleaved
                    into the next block's jt loop (or flushed at the end).
                    step 0: denominator transposes + reciprocal; 1: broadcast
                    + normalize; 2..5: two po pairs each."""
                    i0 = pb * 2048 + pib * 1024
                    key = f"{pb}_{pib}"
                    if step == 0:
                        # colsum rows -> partitions, reciprocal on 128 lanes,
                        # transpose back to [1, 1024] rows
                        pt = pvps.tile([128, 16], F32, tag="pv0", name=f"pt{key}")
                        for h in range(2):
                            for blk in range(8):
                                c = h * 8 + blk
                                nc.tensor.transpose(
                                    pt[:, c : c + 1],
                                    csd[key][h][0:1, blk * 128 : (blk + 1) * 128],
                                    identf[0:1, 0:1],
                                )
                        rT = sp.tile([128, 16], F32, tag="rT", name=f"rT{key}")
                        nc.vector.reciprocal(rT[:], pt[:])
                        pr = pvps.tile([16, 128], F32, tag="pv1", name=f"pr{key}")
                        nc.tensor.transpose(pr[:], rT[:], identf[:])
                        prs = sp.tile([16, 128], F32R, tag="prs", name=f"prs{key}")
                        nc.vector.tensor_copy(prs[:], pr[:])
                        # gather each head's 8 contiguous partition-rows into a
                        # [1, 1024] rhs row for the K=1 broadcast matmul
                        r2 = [
                            sp.tile([1, 1024], F32R, tag=f"r{h}", name=f"r{key}_{h}")
                            for h in range(2)
                        ]
                        for h in range(2):
                            nc.sync.dma_start(
                                r2[h][0:1, :], prs[h * 8 : (h + 1) * 8, :]
                            )
                        rcp[key] = r2
                    elif step == 1:
                        for h in range(2):
                            rbc = pvps.tile(
                                [64, 1024], F32, tag=f"pv{h}", name=f"rbc{key}_{h}"
                            )
                            for ih in range(2):
                                nc.tensor.matmul(
                                    rbc[:, ih * 512 : (ih + 1) * 512],
                                    ones1[:],
                                    rcp[key][h][0:1, ih * 512 : (ih + 1) * 512],
                                    start=True,
                                    stop=True,
                                )
                            rbs = sp.tile([64, 1024], F32, tag=f"rbs{h}", name=f"rbs{key}_{h}")
                            nc.vector.tensor_copy(rbs[:], rbc[:])
                            nc.vector.tensor_tensor(
                                attnT[h * 64 : (h + 1) * 64, i0 : i0 + 1024],
                                unorm[key][h][:],
                                rbs[:],
                                mult,
                            )
                    else:
                        for k in range(2):
                            tg = (step - 2) * 2 + k
                            row = i0 + tg * 128
                            po = pvps.tile(
                                [128, 1024], F32, tag=f"pv{tg % 2}", name=f"po{key}_{tg}"
                            )
                            for oh in range(2):
                                nc.tensor.matmul(
                                    po[:, oh * 512 : (oh + 1) * 512],
                                    attnT[:, row : row + 128],
                                    wo_g[:, oh * 512 : (oh + 1) * 512],
                                    start=True,
                                    stop=True,
                                )
                            ob = osp.tile(
                                [128, 1024], F32, tag="ob", bufs=4, name=f"ob{key}_{tg}"
                            )
                            nc.vector.tensor_copy(ob[:], po[:])
                            dmae = nc.sync if tg % 2 == 0 else nc.gpsimd
                            dmae.dma_start(out_d[row : row + 128, :], ob[:])

                unorm = {}
                rcp = {}
                csd = {}
                prev = None
                for b, ib in blocks:
                    key = f"{b}_{ib}"
                    i0 = b * 2048 + ib * 1024
                    # allocated lazily at the first emit_pv so the pv-ring
                    # order is: prev block's pv -> prev's rbc/po -> ours
                    pv = []
                    e_pend = []

                    def emit_pv(jt, key=key, b=b, pv=pv, e_pend=e_pend):
                        if not pv:
                            pv.extend(
                                pvps.tile([65, 1024], F32, tag=f"pv{h}", name=f"pv{key}_{h}")
                                for h in range(2)
                            )
                        eh = e_pend.pop(0)
                        jv = (b * 16 + jt) * 65
                        for h, vv in enumerate((v1, v2)):
                            for ih in range(2):
                                nc.tensor.matmul(
                                    pv[h][:, ih * 512 : (ih + 1) * 512],
                                    vv[:, jv : jv + 65],
                                    eh[h][:, ih * 512 : (ih + 1) * 512],
                                    start=(jt == 0),
                                    stop=(jt == 15),
                                )

                    for jt in range(16):
                        j0 = b * 2048 + jt * 128
                        s_h = [
                            sps.tile([128, 1024], F32, tag=f"s{h}", name=f"s{key}_{jt}_{h}")
                            for h in range(2)
                        ]
                        e_h = [
                            ep.tile([128, 1024], BF16, tag=f"e{h}", bufs=LAG + 2,
                                    name=f"e{key}_{jt}_{h}")
                            for h in range(2)
                        ]
                        # h-major so h0's S+exp only gate on ACT_h0(jt-1):
                        # the two heads' ACTs ping-pong and ScalarE stays busy
                        for h in range(2):
                            for ih in range(2):
                                nc.tensor.matmul(
                                    s_h[h][:, ih * 512 : (ih + 1) * 512],
                                    ks[h][:, j0 : j0 + 128],
                                    qs[h][:, i0 + ih * 512 : i0 + (ih + 1) * 512],
                                    start=True,
                                    stop=True,
                                )
                            nc.scalar.activation(e_h[h][:], s_h[h][:], Exp, scale=SCALE)
                        e_pend.append(e_h)
                        # previous block's norm + out-projection, spread out;
                        # emitted before the lagged PV so the po tiles precede
                        # this block's pv tiles in the psum rings
                        if prev is not None:
                            if jt == 1:
                                emit_boundary(prev[0], prev[1], 0)
                            elif 3 <= jt <= 7:
                                emit_boundary(prev[0], prev[1], jt - 2)
                        if len(e_pend) > LAG:
                            emit_pv(jt - LAG)
                    for jt in range(16 - LAG, 16):
                        emit_pv(jt)

                    # denominator rows + unnormalized attn-out to SBUF; these
                    # free the pv psum banks (norm continues next block)
                    csd[key] = []
                    for h in range(2):
                        cs = sp.tile([1, 1024], F32, tag=f"cs{h}", name=f"cs{key}_{h}")
                        nc.vector.tensor_copy(cs[:], pv[h][64:65, :])
                        csd[key].append(cs)
                    unorm[key] = [
                        sp.tile([64, 1024], F32, tag=f"un{h}", name=f"un{key}_{h}")
                        for h in range(2)
                    ]
                    for h in range(2):
                        nc.vector.tensor_copy(unorm[key][h][:], pv[h][0:64, :])
                    prev = (b, ib)

                # flush the last block's norm + out-projection
                for step in range(6):
                    emit_boundary(prev[0], prev[1], step)

    nc.compile()
    return nc


_NC = None


def _get_nc():
    global _NC
    if _NC is None:
        _NC = build_nc()
    return _NC


def _gate(mask):
    """Exact jax fp32 gate: sigmoid(m) > 0.5 (matches reference rounding)."""
    mask = np.asarray(mask, dtype=np.float32)
    return (np.float32(1.0) / (np.float32(1.0) + np.exp(-mask))) > np.float32(0.5)


def make_in_maps(x, qkv_weight, qkv_weight_mask, out_weight, out_weight_mask):
    import ml_dtypes

    bf16 = ml_dtypes.bfloat16
    x = np.asarray(x, dtype=np.float32)
    wqkv = np.where(_gate(qkv_weight_mask), np.asarray(qkv_weight, np.float32), 0.0)
    wo = np.where(_gate(out_weight_mask), np.asarray(out_weight, np.float32), 0.0)

    xT = np.ascontiguousarray(x.reshape(T, DIM).T).astype(bf16)
    in_maps = []
    for c in range(NCORES):
        r0 = c * DV
        sl = slice(r0, r0 + DV)
        w_shard = np.concatenate(
            [wqkv[sl], wqkv[DIM + r0 : DIM + r0 + DV], wqkv[2 * DIM + r0 : 2 * DIM + r0 + DV]],
            axis=0,
        )  # [384, 1024] rows = (q | k | v) for this core's 2 heads
        in_maps.append(
            {
                "xT": xT,
                "wqkvT": np.ascontiguousarray(w_shard.T).astype(bf16),
                "woT": np.ascontiguousarray(wo[:, sl].T).astype(bf16),
            }
        )
    return in_maps


LAST_RESULTS = None  # BassKernelResults of the most recent run (for profiling)


def kernel(
    x,
    qkv_weight,
    qkv_weight_mask,
    out_weight,
    out_weight_mask,
    out_bias,
    out_bias_mask,
    _trace=False,
    _tmpdir=None,
):
    global LAST_RESULTS
    from concourse.bass_utils import run_bass_kernel_spmd

    nc = _get_nc()
    in_maps = make_in_maps(x, qkv_weight, qkv_weight_mask, out_weight, out_weight_mask)
    res = run_bass_kernel_spmd(
        nc, in_maps, list(range(NCORES)), trace=_trace, tmpdir=_tmpdir
    )
    LAST_RESULTS = res
    out = np.zeros((T, DIM), dtype=np.float32)
    for r in res.results:
        out += r["out"]
    out_bias = np.asarray(out_bias, dtype=np.float32)
    out += np.where(_gate(out_bias_mask), out_bias, np.float32(0.0))[None, :]
    return out.reshape(B, N, DIM)


# revision 28
# speedup vs baseline: 1.6055x; 1.0421x over previous
"""Trainium2 Bass kernel for nn_Attention_41704132444382.

Masked-linear QKV projection + 16-head attention + masked-linear output
projection, tensor-parallel over heads across 8 NeuronCores (2 heads/core).

v2 design (ScalarE-exp is the roofline: ~128us of exp streaming):
  - Host: gates both masked-linear weights (sigmoid(m)>0.5), transposes x,
    casts x / wqkv / wo to bf16 (wqkv/wo values are +-c, near-exact in bf16).
  - QKV: xq bf16 tiles [128, 4096] x 8 kt-chunks; lhsT = gated wqkv bf16
    (FWL weight loads); psum [128,512] chains; q/k evacuated by ScalarE
    (Copy -> f32r), v by DVE (cast -> bf16). V^T PE-transposed (bf16) to
    v1/v2 [t, dv|1] tiles with a ones column at stride 65 (PV then yields
    attn-out^T AND the softmax denominator in one accumulation chain).
  - Attention per 1024-query block, h-offset pipeline: per key-tile jt,
    S^T = kT.T @ qT per head into s_h [128,1024] (2 psum banks); one
    1024-wide exp ACT per head (scale=1/32) -> e_h bf16; PV lags LAG
    key-tiles behind (e-ring depth covers it) so block-boundary work can
    drain the pv psum rings without stalling ScalarE.
  - Softmax denominators: pv row 64 -> [1,1024] copy, DVE reciprocal
    (f32r out), broadcast to [64,1024] via K=1 ones-matmul, normalize
    attnT with one tensor_tensor per head.
  - Output projection: lhsT = attnT bf16 (FWL), po pairs [128,1024] in
    the pv psum rings at block boundaries, DVE evac, DMA from SBUF.
"""

import os
import sys

import numpy as np

sys.path.insert(0, "/opt/trn_rl_repo")

import concourse.bass as bass
import concourse.mybir as mybir
from concourse import bacc
from concourse.masks import make_identity
from concourse.tile import TileContext

DIM = 1024
HEADS = 16
B = 2
N = 2048
T = B * N  # 4096 flattened tokens
NCORES = 8
HPC = HEADS // NCORES  # 2 heads per core
DV = HPC * 64  # 128 head-dims per core
SCALE = DIM ** (-0.5)  # 1/32
# PV runs LAG key-tiles behind exp so block-boundary work (norm broadcast +
# out-projection) can drain through the pv psum rings without stalling the
# ScalarE exp pipeline; the e-ring depth covers the lag.
LAG = 7

F32 = mybir.dt.float32
F32R = mybir.dt.float32r
BF16 = mybir.dt.bfloat16

Copy = mybir.ActivationFunctionType.Copy
Exp = mybir.ActivationFunctionType.Exp
mult = mybir.AluOpType.mult


def build_nc():
    nc = bacc.Bacc("TRN2", target_bir_lowering=True)
    xT_d = nc.declare_dram_parameter("xT", [DIM, T], BF16, isOutput=False)
    wqkvT_d = nc.declare_dram_parameter("wqkvT", [DIM, 384], BF16, isOutput=False)
    woT_d = nc.declare_dram_parameter("woT", [DV, DIM], BF16, isOutput=False)
    out_d = nc.declare_dram_parameter("out", [T, DIM], F32, isOutput=True)

    with TileContext(nc) as tc:
        with tc.tile_pool(name="persist", bufs=1) as pp:
            # S runs as K=128 all-bf16 matmuls (FWL weight loads keep PE array
            # duty high -> HAM stays at 2.4 GHz) with split-precision q in the
            # otherwise-idle half of the array: qs rows 0-63 = bf16(q), rows
            # 64-127 = bf16(q - bf16(q)); ks duplicates k in both halves, so
            # k.T q accumulates the hi and lo products -> q is fp32-exact.
            qs = [pp.tile([128, T], BF16, name=f"qs{h}") for h in range(HPC)]
            ks = [pp.tile([128, T], BF16, name=f"ks{h}") for h in range(HPC)]
            v1 = pp.tile([128, 32 * 65], BF16)  # [t-part, (jt, dv|1)] head 0
            v2 = pp.tile([128, 32 * 65], BF16)  # head 1
            attnT = pp.tile([128, T], BF16)  # [dv-part, t] normalized
            wo_g = pp.tile([128, DIM], BF16)
            ident = pp.tile([128, 128], BF16)
            identf = pp.tile([128, 128], F32)
            ones1 = pp.tile([1, 64], F32R)

            make_identity(nc, ident[:])
            make_identity(nc, identf[:])
            ones_f = pp.tile([128, 64], F32)
            nc.vector.memset(ones_f[:], 1.0)
            nc.vector.tensor_copy(ones1[:], ones_f[0:1, :])
            ones32 = pp.tile([128, 32], BF16)
            nc.vector.tensor_copy(ones32[:], ones_f[:, 0:32])
            # ones column at slot 64 of each 65-wide block of v1/v2; V
            # evacuations only write cols 0..63 of each block.
            for vv in (v1, v2):
                nc.vector.tensor_copy(
                    vv[:].rearrange("p (j c) -> p j c", c=65)[:, :, 64:65],
                    ones32[:].rearrange("p (j c) -> p j c", c=1),
                )
            # preload the exp activation table while DMAs run
            junk = pp.tile([1, 32], F32)
            nc.vector.memset(junk[:], 0.0)
            junk2 = pp.tile([1, 32], F32)
            nc.scalar.activation(junk2[:], junk[:], Exp)

            nc.sync.dma_start(wo_g[:], woT_d[:])

            # ---------- Phase 1: QKV projection (+ V^T transpose) ----------
            with (
                tc.tile_pool(name="ph1", bufs=1) as p1,
                tc.tile_pool(name="qkv_ps", bufs=4, space="PSUM") as qkps,
            ):
                wqkv_g = p1.tile([128, 8 * 384], BF16)  # [k-part, (kt, o)]
                nc.sync.dma_start(
                    wqkv_g[:].rearrange("p (kt o) -> p kt o", kt=8),
                    wqkvT_d[:].rearrange("(kt p) o -> p kt o", p=128),
                )
                xq = [p1.tile([128, T], BF16, name=f"xq{i}") for i in range(8)]
                vT = p1.tile([128, T], BF16)
                # x arrives in t-quarter chunks so quarter-0 compute starts
                # after ~1/4 of the x traffic
                dmae = [nc.sync, nc.gpsimd, nc.scalar]
                for q in range(4):
                    for kt in range(8):
                        dmae[kt % 3].dma_start(
                            xq[kt][:, q * 1024 : (q + 1) * 1024],
                            xT_d[kt * 128 : (kt + 1) * 128, q * 1024 : (q + 1) * 1024],
                        )

                sub = mybir.AluOpType.subtract
                for q in range(4):
                    # v first so transposes can interleave with q/k matmuls
                    for ot in (2, 1, 0):
                        for th in range(2):
                            ps = qkps.tile([128, 512], F32, tag="qk")
                            for kt in range(8):
                                nc.tensor.matmul(
                                    ps[:],
                                    wqkv_g[:, kt * 384 + ot * 128 : kt * 384 + (ot + 1) * 128],
                                    xq[kt][:, q * 1024 + th * 512 : q * 1024 + (th + 1) * 512],
                                    start=(kt == 0),
                                    stop=(kt == 7),
                                )
                            col = q * 1024 + th * 512
                            cs_ = slice(col, col + 512)
                            if ot == 2:
                                nc.vector.tensor_copy(vT[:, cs_], ps[:])
                            elif ot == 0:  # q: hi = bf16(q), lo = q - hi
                                for hh in range(2):
                                    php = ps[hh * 64 : (hh + 1) * 64, :]
                                    nc.scalar.activation(qs[hh][0:64, cs_], php, Copy)
                                    nc.vector.tensor_tensor(
                                        qs[hh][64:128, cs_], php, qs[hh][0:64, cs_], sub
                                    )
                            else:  # k: duplicated into both array halves
                                for hh in range(2):
                                    php = ps[hh * 64 : (hh + 1) * 64, :]
                                    nc.scalar.activation(ks[hh][0:64, cs_], php, Copy)
                                    nc.vector.tensor_copy(ks[hh][64:128, cs_], ks[hh][0:64, cs_])
                    for tj in range(8):  # V^T -> v1/v2 for this quarter
                        jt = q * 8 + tj
                        ptv = qkps.tile([128, 128], BF16, tag="vt", bufs=2)
                        nc.tensor.transpose(
                            ptv[:], vT[:, jt * 128 : (jt + 1) * 128], ident[:]
                        )
                        nc.vector.tensor_copy(v1[:, jt * 65 : jt * 65 + 64], ptv[:, 0:64])
                        nc.vector.tensor_copy(v2[:, jt * 65 : jt * 65 + 64], ptv[:, 64:128])

            # ---------- Phase 2: attention ----------
            with (
                tc.tile_pool(name="esb", bufs=1) as ep,
                tc.tile_pool(name="small", bufs=1) as sp,
                tc.tile_pool(name="osb", bufs=1) as osp,
                tc.tile_pool(name="s_ps", bufs=1, space="PSUM") as sps,
                tc.tile_pool(name="pv_ps", bufs=1, space="PSUM") as pvps,
            ):
                blocks = [(b, ib) for b in range(B) for ib in range(2)]

                def emit_boundary(pb, pib, step):
                    """Norm + out-projection for block (pb, pib), interleaved
                    into the next block's jt loop (or flushed at the end).
                    step 0: denominator transposes + reciprocal; 1: broadcast
                    + normalize; 2..5: two po pairs each."""
                    i0 = pb * 2048 + pib * 1024
                    key = f"{pb}_{pib}"
                    if step == 0:
                        # colsum rows -> partitions, reciprocal on 128 lanes,
                        # transpose back to [1, 1024] rows
                        pt = pvps.tile([128, 16], F32, tag="pv0", name=f"pt{key}")
                        for h in range(2):
                            for blk in range(8):
                                c = h * 8 + blk
                                nc.tensor.transpose(
                                    pt[:, c : c + 1],
                                    csd[key][h][0:1, blk * 128 : (blk + 1) * 128],
                                    identf[0:1, 0:1],
                                )
                        rT = sp.tile([128, 16], F32, tag="rT", name=f"rT{key}")
                        nc.vector.reciprocal(rT[:], pt[:])
                        pr = pvps.tile([16, 128], F32, tag="pv1", name=f"pr{key}")
                        nc.tensor.transpose(pr[:], rT[:], identf[:])
                        prs = sp.tile([16, 128], F32R, tag="prs", name=f"prs{key}")
                        nc.vector.tensor_copy(prs[:], pr[:])
                        # gather each head's 8 contiguous partition-rows into a
                        # [1, 1024] rhs row for the K=1 broadcast matmul
                        r2 = [
                            sp.tile([1, 1024], F32R, tag=f"r{h}", name=f"r{key}_{h}")
                            for h in range(2)
                        ]
                        for h in range(2):
                            nc.sync.dma_start(
                                r2[h][0:1, :], prs[h * 8 : (h + 1) * 8, :]
                            )
                        rcp[key] = r2
                    elif step == 1:
                        for h in range(2):
                            rbc = pvps.tile(
                                [64, 1024], F32, tag=f"pv{h}", name=f"rbc{key}_{h}"
                            )
                            for ih in range(2):
                                nc.tensor.matmul(
                                    rbc[:, ih * 512 : (ih + 1) * 512],
                                    ones1[:],
                                    rcp[key][h][0:1, ih * 512 : (ih + 1) * 512],
                                    start=True,
                                    stop=True,
                                )
                            rbs = sp.tile([64, 1024], F32, tag=f"rbs{h}", name=f"rbs{key}_{h}")
                            nc.vector.tensor_copy(rbs[:], rbc[:])
                            nc.vector.tensor_tensor(
                                attnT[h * 64 : (h + 1) * 64, i0 : i0 + 1024],
                                unorm[key][h][:],
                                rbs[:],
                                mult,
                            )
                    else:
                        for k in range(2):
                            tg = (step - 2) * 2 + k
                            row = i0 + tg * 128
                            po = pvps.tile(
                                [128, 1024], F32, tag=f"pv{tg % 2}", name=f"po{key}_{tg}"
                            )
                            for oh in range(2):
                                nc.tensor.matmul(
                                    po[:, oh * 512 : (oh + 1) * 512],
                                    attnT[:, row : row + 128],
                                    wo_g[:, oh * 512 : (oh + 1) * 512],
                                    start=True,
                                    stop=True,
                                )
                            ob = osp.tile(
                                [128, 1024], F32, tag="ob", bufs=6, name=f"ob{key}_{tg}"
                            )
                            # in the tail (flush) ScalarE is idle: split evacs
                            if flush and tg % 2 == 1:
                                nc.scalar.activation(ob[:], po[:], Copy)
                            else:
                                nc.vector.tensor_copy(ob[:], po[:])
                            dmae = nc.sync if tg % 2 == 0 else nc.gpsimd
                            dmae.dma_start(out_d[row : row + 128, :], ob[:])

                unorm = {}
                rcp = {}
                csd = {}
                flush = False
                pend = []  # closures: lagged PV groups + block-end evacuations
                # drain to a per-jt target queue depth: PE slack per jt fits
                # ~1.2 PV groups, so each block's PV tail spills into the next
                # block's early key-tiles (norm/po occupy jt5-8's slack); the
                # targets keep pv allocations at jt9, after the previous
                # block's norm/po tiles in the psum rings
                TARGET = [9, 8, 7, 6, 5, 6, 7, 8, 9, 9, 9, 9, 9, 9, 9, 9]
                prev = None
                for b, ib in blocks:
                    key = f"{b}_{ib}"
                    i0 = b * 2048 + ib * 1024
                    # allocated lazily at the first emit_pv so the pv-ring
                    # order is: prev block's pv -> prev's rbc/po -> ours
                    pv = []
                    e_pend = []

                    def emit_pv(jt, key=key, b=b, pv=pv, e_pend=e_pend):
                        if not pv:
                            pv.extend(
                                pvps.tile([65, 1024], F32, tag=f"pv{h}", name=f"pv{key}_{h}")
                                for h in range(2)
                            )
                        eh = e_pend.pop(0)
                        jv = (b * 16 + jt) * 65
                        for h, vv in enumerate((v1, v2)):
                            for ih in range(2):
                                nc.tensor.matmul(
                                    pv[h][:, ih * 512 : (ih + 1) * 512],
                                    vv[:, jv : jv + 65],
                                    eh[h][:, ih * 512 : (ih + 1) * 512],
                                    start=(jt == 0),
                                    stop=(jt == 15),
                                )

                    def emit_evac(key=key, pv=pv):
                        # denominator rows + unnormalized attn-out to SBUF;
                        # frees the pv psum banks (norm continues next block)
                        csd[key] = []
                        for h in range(2):
                            cs = sp.tile([1, 1024], F32, tag=f"cs{h}", name=f"cs{key}_{h}")
                            nc.vector.tensor_copy(cs[:], pv[h][64:65, :])
                            csd[key].append(cs)
                        unorm[key] = [
                            sp.tile([64, 1024], F32, tag=f"un{h}", name=f"un{key}_{h}")
                            for h in range(2)
                        ]
                        for h in range(2):
                            nc.vector.tensor_copy(unorm[key][h][:], pv[h][0:64, :])

                    for jt in range(16):
                        j0 = b * 2048 + jt * 128
                        s_h = [
                            sps.tile([128, 1024], F32, tag=f"s{h}", name=f"s{key}_{jt}_{h}")
                            for h in range(2)
                        ]
                        e_h = [
                            ep.tile([128, 1024], BF16, tag=f"e{h}", bufs=14,
                                    name=f"e{key}_{jt}_{h}")
                            for h in range(2)
                        ]
                        # h-major so h0's S+exp only gate on ACT_h0(jt-1):
                        # the two heads' ACTs ping-pong and ScalarE stays busy
                        for h in range(2):
                            for ih in range(2):
                                nc.tensor.matmul(
                                    s_h[h][:, ih * 512 : (ih + 1) * 512],
                                    ks[h][:, j0 : j0 + 128],
                                    qs[h][:, i0 + ih * 512 : i0 + (ih + 1) * 512],
                                    start=True,
                                    stop=True,
                                )
                            nc.scalar.activation(e_h[h][:], s_h[h][:], Exp, scale=SCALE)
                        e_pend.append(e_h)
                        pend.append(lambda jt=jt, f=emit_pv: f(jt))

                        # previous block's norm + out-projection, emitted
                        # before this block's pv allocations enter the rings
                        if prev is not None:
                            if jt == 5:
                                emit_boundary(prev[0], prev[1], 0)
                            elif jt == 6:
                                emit_boundary(prev[0], prev[1], 1)
                            elif jt == 7:
                                emit_boundary(prev[0], prev[1], 2)
                                emit_boundary(prev[0], prev[1], 3)
                            elif jt == 8:
                                emit_boundary(prev[0], prev[1], 4)
                                emit_boundary(prev[0], prev[1], 5)
                        while len(pend) > TARGET[jt]:
                            pend.pop(0)()
                    pend.append(emit_evac)
                    prev = (b, ib)

                # drain everything and flush the last block's norm + po
                for f in pend:
                    f()
                pend.clear()
                flush = True
                for step in range(6):
                    emit_boundary(prev[0], prev[1], step)

    nc.compile()
    return nc


_NC = None


def _get_nc():
    global _NC
    if _NC is None:
        _NC = build_nc()
    return _NC


def _gate(mask):
    """Exact jax fp32 gate: sigmoid(m) > 0.5 (matches reference rounding)."""
    mask = np.asarray(mask, dtype=np.float32)
    return (np.float32(1.0) / (np.float32(1.0) + np.exp(-mask))) > np.float32(0.5)


def make_in_maps(x, qkv_weight, qkv_weight_mask, out_weight, out_weight_mask):
    import ml_dtypes

    bf16 = ml_dtypes.bfloat16
    x = np.asarray(x, dtype=np.float32)
    wqkv = np.where(_gate(qkv_weight_mask), np.asarray(qkv_weight, np.float32), 0.0)
    wo = np.where(_gate(out_weight_mask), np.asarray(out_weight, np.float32), 0.0)

    xT = np.ascontiguousarray(x.reshape(T, DIM).T).astype(bf16)
    in_maps = []
    for c in range(NCORES):
        r0 = c * DV
        sl = slice(r0, r0 + DV)
        w_shard = np.concatenate(
            [wqkv[sl], wqkv[DIM + r0 : DIM + r0 + DV], wqkv[2 * DIM + r0 : 2 * DIM + r0 + DV]],
            axis=0,
        )  # [384, 1024] rows = (q | k | v) for this core's 2 heads
        in_maps.append(
            {
                "xT": xT,
                "wqkvT": np.ascontiguousarray(w_shard.T).astype(bf16),
                "woT": np.ascontiguousarray(wo[:, sl].T).astype(bf16),
            }
        )
    return in_maps


LAST_RESULTS = None  # BassKernelResults of the most recent run (for profiling)


def kernel(
    x,
    qkv_weight,
    qkv_weight_mask,
    out_weight,
    out_weight_mask,
    out_bias,
    out_bias_mask,
    _trace=False,
    _tmpdir=None,
):
    global LAST_RESULTS
    from concourse.bass_utils import run_bass_kernel_spmd

    nc = _get_nc()
    in_maps = make_in_maps(x, qkv_weight, qkv_weight_mask, out_weight, out_weight_mask)
    res = run_bass_kernel_spmd(
        nc, in_maps, list(range(NCORES)), trace=_trace, tmpdir=_tmpdir
    )
    LAST_RESULTS = res
    out = np.zeros((T, DIM), dtype=np.float32)
    for r in res.results:
        out += r["out"]
    out_bias = np.asarray(out_bias, dtype=np.float32)
    out += np.where(_gate(out_bias_mask), out_bias, np.float32(0.0))[None, :]
    return out.reshape(B, N, DIM)
